# revision 20
# baseline (speedup 1.0000x reference)
"""Weighted 2D Gaussian KDE on 8 Trainium2 NeuronCores (Bass/Tile).

out[b,l] = sum_n w[n] * exp(-||x[b,l] - data[n]||^2 / sigma),  sigma = 3.

Grid-quadrature factorization (exponentially accurate):
  exp(-(s-t)^2/sigma) = F * sum_j exp(-a(s-u_j)^2) * exp(-a(u_j-t)^2)
  over a uniform grid u_j (spacing h, a = 2/sigma, F = h*sqrt(2a/pi)).
  The 2D kernel separates per dim; with the 64x64 moment matrix
  M = F^2 * P1 diag(w) P0^T the KDE is out[c] = q0[:,c]^T M^T q1[:,c].

Device pipeline (per core, locations sharded 16384/core), all-bf16 PE:
  stage A (points, transposed layout): one K=48 matmul produces exp-args for
  4 chunks x 128 points x (both dims' grids); ScalarE exps 8 chunks at once;
  K=64 lo/hi matmul pairs accumulate M into PSUM. The lo/hi split alternates
  row groups 0-63 / 64-127 so every LDWEIGHTS hides under the previous
  matmul (disjoint row groups -> PE pulls the load ahead).
  stage B: per 512-loc strip, arg matmul (weights parked in rows 96+),
  ScalarE exp -> Q (partitions 0-63 dim1, 64-127 dim0), T = M^T q1 (rows
  0-63), r = q0*T on DVE, ones-matmul (rows 64-127) accumulates strip sums
  into one [32, 512] PSUM tile which is the final output layout. T and ones
  alternate disjoint row groups so their weight loads hide too.

fp32-accurate exp arguments come from hi/lo bf16 splits of every product
term (host-precomputed), so the PE streams 1 col/cycle instead of fp32's
LOW_HIGH half rate.
"""

import os
import numpy as np
import ml_dtypes

import concourse.bass as bass
import concourse.tile as tile
from concourse import bacc
from concourse import mybir
from concourse import bass_utils

# ---- problem constants (hardcoded per spec) ----
B, L, D = 2, 65536, 2
NPTS = 16384
NCORES = 8
NLOC = B * L
NSH = NLOC // NCORES      # 16384 locations per core
SIGMA = 3.0
A = 2.0 / SIGMA
H = 0.75
PADG = 3.0
NG = 64
FQ = float(H * np.sqrt(2.0 * A / np.pi))
NSTRIP = 512
NSTRIPS = NSH // NSTRIP   # 32
NPSH = NPTS              # stage A replicated on every core (collectives are
                          # ~60us on this runtime -- measured, not viable)
NCHUNK = NPSH // 128      # 128 point chunks
KA = 12                   # stage A rows per chunk
KA4 = 4 * KA              # 48: 4 chunks per arg matmul
KB = 10                   # stage B contraction rows
NGA = NCHUNK // 8         # 2 stage-A groups (8 chunks / group)
NGB = NSTRIPS // 2        # 16 stage-B iterations (2 strips each)
LAGB = 4                  # strips of argB/exp run-ahead

F32 = mybir.dt.float32
BF16 = mybir.dt.bfloat16
BF = ml_dtypes.bfloat16
AF = mybir.ActivationFunctionType


def _build_core_program(nc: bass.Bass):
    augA = nc.dram_tensor("augA", [KA4, NPSH // 4], BF16, kind="ExternalInput").ap()
    uwa = nc.dram_tensor("uwa", [KA4, 512], BF16, kind="ExternalInput").ap()
    augB = nc.dram_tensor("augB", [KB, NSH], BF16, kind="ExternalInput").ap()
    uwb = nc.dram_tensor("uwb", [KB, 128], BF16, kind="ExternalInput").ap()
    ubias = nc.dram_tensor("ubias", [128, 1], F32, kind="ExternalInput").ap()
    onesw = nc.dram_tensor("onesw", [64, 32 * 32], BF16, kind="ExternalInput").ap()
    out = nc.dram_tensor("out", [NSH], F32, kind="ExternalOutput").ap()

    with tile.TileContext(nc) as tc:
        with (
            tc.tile_pool(name="const", bufs=1) as const,
            tc.tile_pool(name="sbP", bufs=3) as sbP,
            tc.tile_pool(name="sbQ", bufs=5) as sbQ,
            tc.tile_pool(name="sbR", bufs=4) as sbR,
            tc.tile_pool(name="sbO", bufs=1) as sbO,
            tc.tile_pool(name="psArg", bufs=2, space="PSUM") as psArg,
            tc.tile_pool(name="psS", bufs=1, space="PSUM") as psS,
        ):
            augA_sb = const.tile([KA4, NPSH // 4], BF16)
            nc.sync.dma_start(out=augA_sb, in_=augA)
            uwa_sb = const.tile([KA4, 512], BF16)
            nc.sync.dma_start(out=uwa_sb, in_=uwa)
            augB_sb = const.tile([KB, NSH], BF16)
            nc.sync.dma_start(out=augB_sb, in_=augB)
            uwb_sb = const.tile([KB, 128], BF16)
            nc.sync.dma_start(out=uwb_sb, in_=uwb)
            ubias_sb = const.tile([128, 1], F32)
            nc.sync.dma_start(out=ubias_sb, in_=ubias)
            onesw_sb = const.tile([128, 32 * 32], BF16)
            nc.sync.dma_start(out=onesw_sb[64:128, :], in_=onesw)
            m2t = const.tile([64, 64], BF16)

            outps = psS.tile([32, NSTRIP], F32, tag="out", bufs=1)

            # ---------------- stage A: moment matrix M ----------------
            # the hi/lo row-group accumulators live in a scoped pool whose
            # banks are recycled for stage B's T tiles after M is folded
            psM = tc.tile_pool(name="psM", bufs=1, space="PSUM")
            psM_pool = psM.__enter__()
            m2ps_hi = psM_pool.tile([64, 64], F32, tag="m2hi", bufs=1)
            m2ps_lo = psM_pool.tile([64, 64], F32, tag="m2lo", bufs=1)
            p4 = [None] * NGA

            def emit_argA(g):
                ps = psArg.tile([128, 1024], F32, tag="arg")
                for h in range(2):
                    m = 2 * g + h
                    nc.tensor.matmul(
                        ps[:, h * 512 : (h + 1) * 512],
                        lhsT=augA_sb[:, m * 128 : (m + 1) * 128],
                        rhs=uwa_sb,
                        start=True, stop=True, skip_group_check=True,
                    )
                p = sbP.tile([128, 1024], BF16, tag="P")
                nc.scalar.activation(p, ps, AF.Exp)
                p4[g] = p

            def emit_accumA(g):
                p = p4[g]
                for s in range(8):
                    c = g * 8 + s
                    c0 = s * 128
                    # hi then lo: alternating row groups hide every LDW
                    nc.tensor.matmul(
                        m2ps_hi,
                        lhsT=p[64:128, c0 : c0 + 64],
                        rhs=p[64:128, c0 + 64 : c0 + 128],
                        start=(c == 0), stop=(c == NCHUNK - 1),
                        skip_group_check=True,
                    )
                    nc.tensor.matmul(
                        m2ps_lo,
                        lhsT=p[0:64, c0 : c0 + 64],
                        rhs=p[0:64, c0 + 64 : c0 + 128],
                        start=(c == 0), stop=(c == NCHUNK - 1),
                        skip_group_check=True,
                    )

            for g in range(NGA + 2):
                if g < NGA:
                    emit_argA(g)
                if g >= 2:
                    emit_accumA(g - 2)
            # combine hi/lo accumulators, fold the 2D quadrature factor,
            # then AllReduce the partial moment matrix across the 8 cores
            m2fh = const.tile([64, 64], F32)
            nc.scalar.mul(m2fh, m2ps_hi, FQ * FQ)
            m2fl = const.tile([64, 64], F32)
            nc.scalar.mul(m2fl, m2ps_lo, FQ * FQ)
            nc.vector.tensor_add(m2t, m2fh, m2fl)
            psM.__exit__(None, None, None)
            psT_cm = tc.tile_pool(name="psT", bufs=3, space="PSUM")
            psT = psT_cm.__enter__()

            # ---------------- stage B: per-location evaluation ----------------
            qt = [None] * NGB

            def emit_argB(i):
                ps = psArg.tile([128, 1024], F32, tag="arg")
                for h in range(2):
                    t = 2 * i + h
                    nc.tensor.matmul(
                        ps[:, h * 512 : (h + 1) * 512],
                        lhsT=uwb_sb,
                        rhs=augB_sb[:, t * NSTRIP : (t + 1) * NSTRIP],
                        start=True, stop=True, skip_group_check=True,
                    )
                q = sbQ.tile([128, 1024], BF16, tag="Q")
                nc.scalar.activation(q, ps, AF.Exp, bias=ubias_sb)
                qt[i] = q

            rts = [None] * NSTRIPS

            def emit_T(t):
                q = qt[t // 2]
                sl = slice((t % 2) * NSTRIP, (t % 2 + 1) * NSTRIP)
                tp = psT.tile([128, NSTRIP], F32, tag="T")
                nc.tensor.matmul(
                    tp[64:128, :], lhsT=m2t, rhs=q[0:64, sl],
                    start=True, stop=True,
                )
                r = sbR.tile([128, NSTRIP], BF16, tag="r")
                nc.vector.tensor_mul(r[64:128, :], q[64:128, sl], tp[64:128, :])
                rts[t] = r

            def emit_ones(t):
                # lagged one strip behind T so its wait on the DVE mul never
                # blocks the next T matmul in the PE's strict FIFO
                nc.tensor.matmul(
                    outps,
                    lhsT=onesw_sb[64:128, t * 32 : (t + 1) * 32],
                    rhs=rts[t][64:128, :],
                    start=(t == 0), stop=(t == NSTRIPS - 1),
                    skip_group_check=True,
                )

            for t in range(NSTRIPS + LAGB + 1):
                if 0 <= t - LAGB < NSTRIPS:
                    emit_T(t - LAGB)
                if 0 <= t - LAGB - 1 < NSTRIPS:
                    emit_ones(t - LAGB - 1)
                # argB after ones: its weight rows (0-9) are disjoint from the
                # in-flight ones matmul (rows 64-127), so it overlaps it
                if t < NSTRIPS and t % 2 == 0:
                    emit_argB(t // 2)

            outsb = sbO.tile([32, NSTRIP], F32)
            nc.vector.tensor_copy(outsb, outps)
            nc.sync.dma_start(
                out=out.rearrange("(p f) -> p f", p=32), in_=outsb
            )
            psT_cm.__exit__(None, None, None)
    return nc


_CACHE = {}
LAST_RESULTS = None


def _get_nc():
    if "nc" not in _CACHE:
        nc = bacc.Bacc("TRN2", target_bir_lowering=False, debug=False, num_devices=NCORES)
        _build_core_program(nc)
        nc.compile()
        _CACHE["nc"] = nc
    return _CACHE["nc"]


def _bf(v):
    return np.asarray(v, dtype=BF)


def _split2(v):
    hi = _bf(v)
    lo = _bf(np.asarray(v, np.float32) - hi.astype(np.float32))
    return hi, lo


def _host_prep(x, data, weights):
    lo = float(min(x.min(), data.min())) - PADG
    hi = float(max(x.max(), data.max())) + PADG
    ng = int(np.ceil((hi - lo) / H)) + 1
    assert ng <= NG, f"grid {ng} exceeds padded size {NG}"
    u = np.full(NG, -1000.0)
    u[:ng] = lo + np.arange(ng) * H

    w1, w1f = _split2(2.0 * A * u)
    v1, v1f = _split2(-A * u * u)
    # jcat layout: cols/partitions 0-63 = dim1 (q1 side), 64-127 = dim0
    m1 = np.zeros(128); m1[:64] = 1.0
    m0 = np.zeros(128); m0[64:] = 1.0
    w1c = np.tile(w1.astype(np.float64), 2)
    w1fc = np.tile(w1f.astype(np.float64), 2)
    v1c = np.tile(v1.astype(np.float64), 2)
    v1fc = np.tile(v1f.astype(np.float64), 2)
    ubias = np.tile(-A * u * u, 2).astype(np.float32)[:, None]

    # stage A: per-point aug rows, batched 4 chunks per matmul
    d0 = data[:, 0].astype(np.float64)
    d1 = data[:, 1].astype(np.float64)
    lnw = np.maximum(
        np.log(np.maximum(weights.astype(np.float64), 1e-300)), -200.0
    )
    d0c, d0f = _split2(d0)
    d1c, d1f = _split2(d1)
    e0h, e0l = _split2(-A * d0 * d0 + lnw)
    e1h, e1l = _split2(-A * d1 * d1)
    one = np.ones(NPTS)
    augA = np.stack(
        [d0c, d0f, d0c, e0h, e0l, d1c, d1f, d1c, e1h, e1l, one, one]
    ).astype(np.float32)  # [12, NPTS]
    uwa1 = np.stack([
        w1c * m0, w1c * m0, w1fc * m0, m0, m0,
        w1c * m1, w1c * m1, w1fc * m1, m1, m1,
        v1c, v1fc,
    ]).astype(np.float32)  # [12, 128]
    # augA4[12*s + q, m*128 + p] = augA[q, (4m+s)*128 + p]
    a = augA.reshape(KA, NPTS // 512, 4, 128)        # [q, m, s, p]
    augA4 = _bf(a.transpose(2, 0, 1, 3).reshape(KA4, NPTS // 4))  # cols: group-major
    # uwa4[12*s + q, 128*s + j] = uwa1[q, j], zero elsewhere
    uwa4 = np.zeros((KA4, 512), np.float32)
    for s in range(4):
        uwa4[s * KA : (s + 1) * KA, s * 128 : (s + 1) * 128] = uwa1
    uwa4 = _bf(uwa4)

    uwb = _bf(np.stack([
        w1c * m0, w1c * m0, w1fc * m0, m0, m0,
        w1c * m1, w1c * m1, w1fc * m1, m1, m1,
    ]))
    onesw = np.zeros((64, 32 * 32), np.float64)
    for t in range(32):
        onesw[:, 32 * t + t] = 1.0
    return augA4, uwa4, uwb, _bf(onesw), ubias


def _prep_augB(xsh):
    x0 = xsh[:, 0].astype(np.float64)
    x1 = xsh[:, 1].astype(np.float64)
    xc0, xf0 = _split2(x0)
    xc1, xf1 = _split2(x1)
    x2h0, x2l0 = _split2(-A * x0 * x0)
    x2h1, x2l1 = _split2(-A * x1 * x1)
    return _bf(np.stack(
        [xc0, xf0, xc0, x2h0, x2l0, xc1, xf1, xc1, x2h1, x2l1]
    ))


def kernel(x, data, weights):
    global LAST_RESULTS
    x = np.ascontiguousarray(x, dtype=np.float32)
    data = np.ascontiguousarray(data, dtype=np.float32)
    weights = np.ascontiguousarray(weights, dtype=np.float32)
    assert x.shape == (B, L, D) and data.shape == (NPTS, D)

    augA4, uwa4, uwb, onesw, ubias = _host_prep(x, data, weights)
    xf = x.reshape(NLOC, D)
    in_maps = []
    for c in range(NCORES):
        in_maps.append({
            "augA": augA4,
            "uwa": uwa4,
            "augB": _prep_augB(xf[c * NSH : (c + 1) * NSH]),
            "uwb": uwb,
            "ubias": ubias,
            "onesw": onesw,
        })

    nc = _get_nc()
    res = bass_utils.run_bass_kernel_spmd(
        nc, in_maps, core_ids=list(range(NCORES)),
        trace=bool(os.environ.get("BASS_TRACE")),
    )
    LAST_RESULTS = res
    out = np.concatenate([res.results[c]["out"] for c in range(NCORES)])
    return out.reshape(B, L)


# revision 22
# speedup vs baseline: 1.0000x; 1.0000x over previous
"""Weighted 2D Gaussian KDE on 8 Trainium2 NeuronCores (Bass/Tile).

out[b,l] = sum_n w[n] * exp(-||x[b,l] - data[n]||^2 / sigma),  sigma = 3.

Grid-quadrature factorization (exponentially accurate):
  exp(-(s-t)^2/sigma) = F * sum_j exp(-a(s-u_j)^2) * exp(-a(u_j-t)^2)
  over a uniform grid u_j (spacing h, a = 2/sigma, F = h*sqrt(2a/pi)).
  The 2D kernel separates per dim; with the 64x64 moment matrix
  M = F^2 * P1 diag(w) P0^T the KDE is out[c] = q0[:,c]^T M^T q1[:,c].

Device pipeline (per core, locations sharded 16384/core), all-bf16 PE:
  stage A (points, transposed layout): one K=48 matmul produces exp-args for
  4 chunks x 128 points x (both dims' grids); ScalarE exps 8 chunks at once;
  K=64 lo/hi matmul pairs accumulate M into PSUM. The lo/hi split alternates
  row groups 0-63 / 64-127 so every LDWEIGHTS hides under the previous
  matmul (disjoint row groups -> PE pulls the load ahead).
  stage B: per 512-loc strip, arg matmul (K=10, rows 0-9), ScalarE exp ->
  Q (partitions 0-63 dim1, 64-127 dim0), T = M^T q1 (rows 0-63), r = q0*T
  on DVE, ones-matmul (rows 64-127) accumulates strip sums into one
  [32, 512] PSUM tile which is the final output layout. T and ones sit in
  disjoint row groups/PSUM banks; the ones-matmul is emitted with a lag so
  its wait on the DVE mul never blocks later T matmuls in the PE FIFO.

fp32-accurate exp arguments come from hi/lo bf16 splits of every product
term (host-precomputed), so the PE streams 1 col/cycle instead of fp32's
LOW_HIGH half rate.
"""

import os
import numpy as np
import ml_dtypes

import concourse.bass as bass
import concourse.tile as tile
from concourse import bacc
from concourse import mybir
from concourse import bass_utils

# ---- problem constants (hardcoded per spec) ----
B, L, D = 2, 65536, 2
NPTS = 16384
NCORES = 8
NLOC = B * L
NSH = NLOC // NCORES      # 16384 locations per core
SIGMA = 3.0
A = 2.0 / SIGMA
H = 0.75
PADG = 3.0
NG = 64
FQ = float(H * np.sqrt(2.0 * A / np.pi))
NSTRIP = 512
NSTRIPS = NSH // NSTRIP   # 32
NPSH = NPTS              # stage A replicated on every core (collectives are
                          # ~60us on this runtime -- measured, not viable)
NCHUNK = NPSH // 128      # 128 point chunks
KA = 12                   # stage A rows per chunk
KA4 = 4 * KA              # 48: 4 chunks per arg matmul
KB = 10                   # stage B contraction rows
NGA = NCHUNK // 8         # 2 stage-A groups (8 chunks / group)
NGB = NSTRIPS // 2        # 16 stage-B iterations (2 strips each)
LAGB = 4                  # strips of argB/exp run-ahead

F32 = mybir.dt.float32
BF16 = mybir.dt.bfloat16
BF = ml_dtypes.bfloat16
AF = mybir.ActivationFunctionType


def _build_core_program(nc: bass.Bass):
    augA = nc.dram_tensor("augA", [KA4, NPSH // 4], BF16, kind="ExternalInput").ap()
    uwa = nc.dram_tensor("uwa", [KA4, 512], BF16, kind="ExternalInput").ap()
    augB = nc.dram_tensor("augB", [KB, NSH], BF16, kind="ExternalInput").ap()
    uwb = nc.dram_tensor("uwb", [KB, 128], BF16, kind="ExternalInput").ap()
    ubias = nc.dram_tensor("ubias", [128, 1], F32, kind="ExternalInput").ap()
    onesw = nc.dram_tensor("onesw", [64, 32 * 32], BF16, kind="ExternalInput").ap()
    out = nc.dram_tensor("out", [NSH], F32, kind="ExternalOutput").ap()

    with tile.TileContext(nc) as tc:
        with (
            tc.tile_pool(name="const", bufs=1) as const,
            tc.tile_pool(name="sbP", bufs=3) as sbP,
            tc.tile_pool(name="sbQ", bufs=5) as sbQ,
            tc.tile_pool(name="sbR", bufs=4) as sbR,
            tc.tile_pool(name="sbO", bufs=1) as sbO,
            tc.tile_pool(name="psArg", bufs=2, space="PSUM") as psArg,
            tc.tile_pool(name="psS", bufs=1, space="PSUM") as psS,
        ):
            augA_sb = const.tile([KA4, NPSH // 4], BF16)
            nc.sync.dma_start(out=augA_sb, in_=augA)
            uwa_sb = const.tile([KA4, 512], BF16)
            nc.sync.dma_start(out=uwa_sb, in_=uwa)
            augB_sb = const.tile([KB, NSH], BF16)
            nc.sync.dma_start(out=augB_sb, in_=augB)
            uwb_sb = const.tile([KB, 128], BF16)
            nc.sync.dma_start(out=uwb_sb, in_=uwb)
            ubias_sb = const.tile([128, 1], F32)
            nc.sync.dma_start(out=ubias_sb, in_=ubias)
            onesw_sb = const.tile([128, 32 * 32], BF16)
            nc.sync.dma_start(out=onesw_sb[64:128, :], in_=onesw)
            m2t = const.tile([64, 64], BF16)

            outps = psS.tile([32, NSTRIP], F32, tag="out", bufs=1)

            # ---------------- stage A: moment matrix M ----------------
            # the hi/lo row-group accumulators live in a scoped pool whose
            # banks are recycled for stage B's T tiles after M is folded
            psM = tc.tile_pool(name="psM", bufs=1, space="PSUM")
            psM_pool = psM.__enter__()
            m2ps_hi = psM_pool.tile([64, 64], F32, tag="m2hi", bufs=1)
            m2ps_lo = psM_pool.tile([64, 64], F32, tag="m2lo", bufs=1)
            p4 = [None] * NGA

            def emit_argA(g):
                ps = psArg.tile([128, 1024], F32, tag="arg")
                for h in range(2):
                    m = 2 * g + h
                    nc.tensor.matmul(
                        ps[:, h * 512 : (h + 1) * 512],
                        lhsT=augA_sb[:, m * 128 : (m + 1) * 128],
                        rhs=uwa_sb,
                        start=True, stop=True, skip_group_check=True,
                    )
                p = sbP.tile([128, 1024], BF16, tag="P")
                nc.scalar.activation(p, ps, AF.Exp)
                p4[g] = p

            def emit_accumA(g):
                p = p4[g]
                for s in range(8):
                    c = g * 8 + s
                    c0 = s * 128
                    # hi then lo: alternating row groups hide every LDW
                    nc.tensor.matmul(
                        m2ps_hi,
                        lhsT=p[64:128, c0 : c0 + 64],
                        rhs=p[64:128, c0 + 64 : c0 + 128],
                        start=(c == 0), stop=(c == NCHUNK - 1),
                        skip_group_check=True,
                    )
                    nc.tensor.matmul(
                        m2ps_lo,
                        lhsT=p[0:64, c0 : c0 + 64],
                        rhs=p[0:64, c0 + 64 : c0 + 128],
                        start=(c == 0), stop=(c == NCHUNK - 1),
                        skip_group_check=True,
                    )

            for g in range(NGA + 2):
                if g < NGA:
                    emit_argA(g)
                if g >= 2:
                    emit_accumA(g - 2)
            # combine hi/lo accumulators, fold the 2D quadrature factor,
            # then AllReduce the partial moment matrix across the 8 cores
            m2fh = const.tile([64, 64], F32)
            nc.scalar.mul(m2fh, m2ps_hi, FQ * FQ)
            m2fl = const.tile([64, 64], F32)
            nc.scalar.mul(m2fl, m2ps_lo, FQ * FQ)
            nc.vector.tensor_add(m2t, m2fh, m2fl)
            psM.__exit__(None, None, None)
            psT_cm = tc.tile_pool(name="psT", bufs=3, space="PSUM")
            psT = psT_cm.__enter__()

            # ---------------- stage B: per-location evaluation ----------------
            qt = [None] * NGB

            def emit_argB(i):
                ps = psArg.tile([128, 1024], F32, tag="arg")
                for h in range(2):
                    t = 2 * i + h
                    nc.tensor.matmul(
                        ps[:, h * 512 : (h + 1) * 512],
                        lhsT=uwb_sb,
                        rhs=augB_sb[:, t * NSTRIP : (t + 1) * NSTRIP],
                        start=True, stop=True, skip_group_check=True,
                    )
                q = sbQ.tile([128, 1024], BF16, tag="Q")
                nc.scalar.activation(q, ps, AF.Exp, bias=ubias_sb)
                qt[i] = q

            rts = [None] * NSTRIPS

            def emit_T(t):
                q = qt[t // 2]
                sl = slice((t % 2) * NSTRIP, (t % 2 + 1) * NSTRIP)
                tp = psT.tile([128, NSTRIP], F32, tag="T")
                nc.tensor.matmul(
                    tp[64:128, :], lhsT=m2t, rhs=q[0:64, sl],
                    start=True, stop=True,
                )
                r = sbR.tile([128, NSTRIP], BF16, tag="r")
                nc.vector.tensor_mul(r[64:128, :], q[64:128, sl], tp[64:128, :])
                rts[t] = r

            def emit_ones(t):
                # lagged one strip behind T so its wait on the DVE mul never
                # blocks the next T matmul in the PE's strict FIFO
                nc.tensor.matmul(
                    outps,
                    lhsT=onesw_sb[64:128, t * 32 : (t + 1) * 32],
                    rhs=rts[t][64:128, :],
                    start=(t == 0), stop=(t == NSTRIPS - 1),
                    skip_group_check=True,
                )

            for t in range(NSTRIPS + LAGB + 1):
                if t < NSTRIPS and t % 2 == 0:
                    emit_argB(t // 2)
                if 0 <= t - LAGB < NSTRIPS:
                    emit_T(t - LAGB)
                if 0 <= t - LAGB - 1 < NSTRIPS:
                    emit_ones(t - LAGB - 1)

            outsb = sbO.tile([32, NSTRIP], F32)
            nc.vector.tensor_copy(outsb, outps)
            nc.sync.dma_start(
                out=out.rearrange("(p f) -> p f", p=32), in_=outsb
            )
            psT_cm.__exit__(None, None, None)
    return nc


_CACHE = {}
LAST_RESULTS = None


def _get_nc():
    if "nc" not in _CACHE:
        nc = bacc.Bacc("TRN2", target_bir_lowering=False, debug=False, num_devices=NCORES)
        _build_core_program(nc)
        nc.compile()
        _CACHE["nc"] = nc
    return _CACHE["nc"]


def _bf(v):
    return np.asarray(v, dtype=BF)


def _split2(v):
    hi = _bf(v)
    lo = _bf(np.asarray(v, np.float32) - hi.astype(np.float32))
    return hi, lo


def _host_prep(x, data, weights):
    lo = float(min(x.min(), data.min())) - PADG
    hi = float(max(x.max(), data.max())) + PADG
    ng = int(np.ceil((hi - lo) / H)) + 1
    assert ng <= NG, f"grid {ng} exceeds padded size {NG}"
    u = np.full(NG, -1000.0)
    u[:ng] = lo + np.arange(ng) * H

    w1, w1f = _split2(2.0 * A * u)
    v1, v1f = _split2(-A * u * u)
    # jcat layout: cols/partitions 0-63 = dim1 (q1 side), 64-127 = dim0
    m1 = np.zeros(128); m1[:64] = 1.0
    m0 = np.zeros(128); m0[64:] = 1.0
    w1c = np.tile(w1.astype(np.float64), 2)
    w1fc = np.tile(w1f.astype(np.float64), 2)
    v1c = np.tile(v1.astype(np.float64), 2)
    v1fc = np.tile(v1f.astype(np.float64), 2)
    ubias = np.tile(-A * u * u, 2).astype(np.float32)[:, None]

    # stage A: per-point aug rows, batched 4 chunks per matmul
    d0 = data[:, 0].astype(np.float64)
    d1 = data[:, 1].astype(np.float64)
    lnw = np.maximum(
        np.log(np.maximum(weights.astype(np.float64), 1e-300)), -200.0
    )
    d0c, d0f = _split2(d0)
    d1c, d1f = _split2(d1)
    e0h, e0l = _split2(-A * d0 * d0 + lnw)
    e1h, e1l = _split2(-A * d1 * d1)
    one = np.ones(NPTS)
    augA = np.stack(
        [d0c, d0f, d0c, e0h, e0l, d1c, d1f, d1c, e1h, e1l, one, one]
    ).astype(np.float32)  # [12, NPTS]
    uwa1 = np.stack([
        w1c * m0, w1c * m0, w1fc * m0, m0, m0,
        w1c * m1, w1c * m1, w1fc * m1, m1, m1,
        v1c, v1fc,
    ]).astype(np.float32)  # [12, 128]
    # augA4[12*s + q, m*128 + p] = augA[q, (4m+s)*128 + p]
    a = augA.reshape(KA, NPTS // 512, 4, 128)        # [q, m, s, p]
    augA4 = _bf(a.transpose(2, 0, 1, 3).reshape(KA4, NPTS // 4))  # cols: group-major
    # uwa4[12*s + q, 128*s + j] = uwa1[q, j], zero elsewhere
    uwa4 = np.zeros((KA4, 512), np.float32)
    for s in range(4):
        uwa4[s * KA : (s + 1) * KA, s * 128 : (s + 1) * 128] = uwa1
    uwa4 = _bf(uwa4)

    uwb = _bf(np.stack([
        w1c * m0, w1c * m0, w1fc * m0, m0, m0,
        w1c * m1, w1c * m1, w1fc * m1, m1, m1,
    ]))
    onesw = np.zeros((64, 32 * 32), np.float64)
    for t in range(32):
        onesw[:, 32 * t + t] = 1.0
    return augA4, uwa4, uwb, _bf(onesw), ubias


def _prep_augB(xsh):
    x0 = xsh[:, 0].astype(np.float64)
    x1 = xsh[:, 1].astype(np.float64)
    xc0, xf0 = _split2(x0)
    xc1, xf1 = _split2(x1)
    x2h0, x2l0 = _split2(-A * x0 * x0)
    x2h1, x2l1 = _split2(-A * x1 * x1)
    return _bf(np.stack(
        [xc0, xf0, xc0, x2h0, x2l0, xc1, xf1, xc1, x2h1, x2l1]
    ))


def kernel(x, data, weights):
    global LAST_RESULTS
    x = np.ascontiguousarray(x, dtype=np.float32)
    data = np.ascontiguousarray(data, dtype=np.float32)
    weights = np.ascontiguousarray(weights, dtype=np.float32)
    assert x.shape == (B, L, D) and data.shape == (NPTS, D)

    augA4, uwa4, uwb, onesw, ubias = _host_prep(x, data, weights)
    xf = x.reshape(NLOC, D)
    in_maps = []
    for c in range(NCORES):
        in_maps.append({
            "augA": augA4,
            "uwa": uwa4,
            "augB": _prep_augB(xf[c * NSH : (c + 1) * NSH]),
            "uwb": uwb,
            "ubias": ubias,
            "onesw": onesw,
        })

    nc = _get_nc()
    res = bass_utils.run_bass_kernel_spmd(
        nc, in_maps, core_ids=list(range(NCORES)),
        trace=bool(os.environ.get("BASS_TRACE")),
    )
    LAST_RESULTS = res
    out = np.concatenate([res.results[c]["out"] for c in range(NCORES)])
    return out.reshape(B, L)


# revision 23
# speedup vs baseline: 1.0045x; 1.0045x over previous
"""Weighted 2D Gaussian KDE on 8 Trainium2 NeuronCores (Bass/Tile).

out[b,l] = sum_n w[n] * exp(-||x[b,l] - data[n]||^2 / sigma),  sigma = 3.

Grid-quadrature factorization (exponentially accurate):
  exp(-(s-t)^2/sigma) = F * sum_j exp(-a(s-u_j)^2) * exp(-a(u_j-t)^2)
  over a uniform grid u_j (spacing h, a = 2/sigma, F = h*sqrt(2a/pi)).
  The 2D kernel separates per dim; with the 64x64 moment matrix
  M = F^2 * P1 diag(w) P0^T the KDE is out[c] = q0[:,c]^T M^T q1[:,c].

Device pipeline (per core, locations sharded 16384/core), all-bf16 PE:
  stage A (points, transposed layout): one K=48 matmul produces exp-args for
  4 chunks x 128 points x (both dims' grids); ScalarE exps 8 chunks at once;
  K=64 lo/hi matmul pairs accumulate M into PSUM. The lo/hi split alternates
  row groups 0-63 / 64-127 so every LDWEIGHTS hides under the previous
  matmul (disjoint row groups -> PE pulls the load ahead).
  stage B: per 512-loc strip, arg matmul (K=10, rows 0-9), ScalarE exp ->
  Q (partitions 0-63 dim1, 64-127 dim0), T = M^T q1 (rows 0-63), r = q0*T
  on DVE, ones-matmul (rows 64-127) accumulates strip sums into one
  [32, 512] PSUM tile which is the final output layout. T and ones sit in
  disjoint row groups/PSUM banks; the ones-matmul is emitted with a lag so
  its wait on the DVE mul never blocks later T matmuls in the PE FIFO.

fp32-accurate exp arguments come from hi/lo bf16 splits of every product
term (host-precomputed), so the PE streams 1 col/cycle instead of fp32's
LOW_HIGH half rate.
"""

import os
import numpy as np
import ml_dtypes

import concourse.bass as bass
import concourse.tile as tile
from concourse import bacc
from concourse import mybir
from concourse import bass_utils

# ---- problem constants (hardcoded per spec) ----
B, L, D = 2, 65536, 2
NPTS = 16384
NCORES = 8
NLOC = B * L
NSH = NLOC // NCORES      # 16384 locations per core
SIGMA = 3.0
A = 2.0 / SIGMA
H = 0.75
PADG = 3.0
NG = 64
FQ = float(H * np.sqrt(2.0 * A / np.pi))
NSTRIP = 512
NSTRIPS = NSH // NSTRIP   # 32
NPSH = NPTS              # stage A replicated on every core (collectives are
                          # ~60us on this runtime -- measured, not viable)
NCHUNK = NPSH // 128      # 128 point chunks
KA = 12                   # stage A rows per chunk
KA4 = 4 * KA              # 48: 4 chunks per arg matmul
KB = 10                   # stage B contraction rows
NGA = NCHUNK // 8         # 2 stage-A groups (8 chunks / group)
NGB = NSTRIPS // 2        # 16 stage-B iterations (2 strips each)
LAGB = 4                  # strips of argB/exp run-ahead

F32 = mybir.dt.float32
BF16 = mybir.dt.bfloat16
BF = ml_dtypes.bfloat16
AF = mybir.ActivationFunctionType


def _build_core_program(nc: bass.Bass):
    augA = nc.dram_tensor("augA", [KA4, NPSH // 4], BF16, kind="ExternalInput").ap()
    uwa = nc.dram_tensor("uwa", [KA4, 512], BF16, kind="ExternalInput").ap()
    augB = nc.dram_tensor("augB", [KB, NSH], BF16, kind="ExternalInput").ap()
    uwb = nc.dram_tensor("uwb", [KB, 128], BF16, kind="ExternalInput").ap()
    ubias = nc.dram_tensor("ubias", [128, 1], F32, kind="ExternalInput").ap()
    onesw = nc.dram_tensor("onesw", [64, 32 * 32], BF16, kind="ExternalInput").ap()
    out = nc.dram_tensor("out", [NSH], F32, kind="ExternalOutput").ap()

    with tile.TileContext(nc) as tc:
        with (
            tc.tile_pool(name="const", bufs=1) as const,
            tc.tile_pool(name="sbP", bufs=4) as sbP,
            tc.tile_pool(name="sbQ", bufs=6) as sbQ,
            tc.tile_pool(name="sbR", bufs=16) as sbR,
            tc.tile_pool(name="sbO", bufs=1) as sbO,
            tc.tile_pool(name="psArg", bufs=2, space="PSUM") as psArg,
            tc.tile_pool(name="psS", bufs=1, space="PSUM") as psS,
        ):
            augA_sb = const.tile([KA4, NPSH // 4], BF16)
            nc.sync.dma_start(out=augA_sb, in_=augA)
            uwa_sb = const.tile([KA4, 512], BF16)
            nc.sync.dma_start(out=uwa_sb, in_=uwa)
            augB_sb = const.tile([KB, NSH], BF16)
            nc.sync.dma_start(out=augB_sb, in_=augB)
            uwb_sb = const.tile([KB, 128], BF16)
            nc.sync.dma_start(out=uwb_sb, in_=uwb)
            ubias_sb = const.tile([128, 1], F32)
            nc.sync.dma_start(out=ubias_sb, in_=ubias)
            onesw_sb = const.tile([128, 32 * 32], BF16)
            nc.sync.dma_start(out=onesw_sb[64:128, :], in_=onesw)
            m2t = const.tile([64, 64], BF16)

            outps = psS.tile([32, NSTRIP], F32, tag="out", bufs=1)

            # ---------------- stage A: moment matrix M ----------------
            # the hi/lo row-group accumulators live in a scoped pool whose
            # banks are recycled for stage B's T tiles after M is folded
            psM = tc.tile_pool(name="psM", bufs=1, space="PSUM")
            psM_pool = psM.__enter__()
            m2ps_hi = psM_pool.tile([64, 64], F32, tag="m2hi", bufs=1)
            m2ps_lo = psM_pool.tile([64, 64], F32, tag="m2lo", bufs=1)
            p4 = [None] * NGA

            def emit_argA(g):
                ps = psArg.tile([128, 1024], F32, tag="arg")
                for h in range(2):
                    m = 2 * g + h
                    nc.tensor.matmul(
                        ps[:, h * 512 : (h + 1) * 512],
                        lhsT=augA_sb[:, m * 128 : (m + 1) * 128],
                        rhs=uwa_sb,
                        start=True, stop=True, skip_group_check=True,
                    )
                p = sbP.tile([128, 1024], BF16, tag="P")
                nc.scalar.activation(p, ps, AF.Exp)
                p4[g] = p

            def emit_accumA(g):
                p = p4[g]
                for s in range(8):
                    c = g * 8 + s
                    c0 = s * 128
                    # hi then lo: alternating row groups hide every LDW
                    nc.tensor.matmul(
                        m2ps_hi,
                        lhsT=p[64:128, c0 : c0 + 64],
                        rhs=p[64:128, c0 + 64 : c0 + 128],
                        start=(c == 0), stop=(c == NCHUNK - 1),
                        skip_group_check=True,
                    )
                    nc.tensor.matmul(
                        m2ps_lo,
                        lhsT=p[0:64, c0 : c0 + 64],
                        rhs=p[0:64, c0 + 64 : c0 + 128],
                        start=(c == 0), stop=(c == NCHUNK - 1),
                        skip_group_check=True,
                    )

            for g in range(NGA + 2):
                if g < NGA:
                    emit_argA(g)
                if g >= 2:
                    emit_accumA(g - 2)
            # combine hi/lo accumulators, fold the 2D quadrature factor,
            # then AllReduce the partial moment matrix across the 8 cores
            m2fh = const.tile([64, 64], F32)
            nc.scalar.mul(m2fh, m2ps_hi, FQ * FQ)
            m2fl = const.tile([64, 64], F32)
            nc.scalar.mul(m2fl, m2ps_lo, FQ * FQ)
            nc.vector.tensor_add(m2t, m2fh, m2fl)
            psM.__exit__(None, None, None)
            psT_cm = tc.tile_pool(name="psT", bufs=3, space="PSUM")
            psT = psT_cm.__enter__()

            # ---------------- stage B: per-location evaluation ----------------
            qt = [None] * NGB

            def emit_argB(i):
                ps = psArg.tile([128, 1024], F32, tag="arg")
                for h in range(2):
                    t = 2 * i + h
                    nc.tensor.matmul(
                        ps[:, h * 512 : (h + 1) * 512],
                        lhsT=uwb_sb,
                        rhs=augB_sb[:, t * NSTRIP : (t + 1) * NSTRIP],
                        start=True, stop=True, skip_group_check=True,
                    )
                q = sbQ.tile([128, 1024], BF16, tag="Q")
                nc.scalar.activation(q, ps, AF.Exp, bias=ubias_sb)
                qt[i] = q

            rts = [None] * NSTRIPS

            def emit_T(t):
                q = qt[t // 2]
                sl = slice((t % 2) * NSTRIP, (t % 2 + 1) * NSTRIP)
                tp = psT.tile([128, NSTRIP], F32, tag="T")
                nc.tensor.matmul(
                    tp[64:128, :], lhsT=m2t, rhs=q[0:64, sl],
                    start=True, stop=True,
                )
                r = sbR.tile([128, NSTRIP], BF16, tag="r")
                nc.vector.tensor_mul(r[64:128, :], q[64:128, sl], tp[64:128, :])
                rts[t] = r

            def emit_ones(t):
                # lagged one strip behind T so its wait on the DVE mul never
                # blocks the next T matmul in the PE's strict FIFO
                nc.tensor.matmul(
                    outps,
                    lhsT=onesw_sb[64:128, t * 32 : (t + 1) * 32],
                    rhs=rts[t][64:128, :],
                    start=(t == 0), stop=(t == NSTRIPS - 1),
                    skip_group_check=True,
                )

            for t in range(NSTRIPS + LAGB + 1):
                if t < NSTRIPS and t % 2 == 0:
                    emit_argB(t // 2)
                if 0 <= t - LAGB < NSTRIPS:
                    emit_T(t - LAGB)
                if 0 <= t - LAGB - 1 < NSTRIPS:
                    emit_ones(t - LAGB - 1)

            outsb = sbO.tile([32, NSTRIP], F32)
            nc.vector.tensor_copy(outsb, outps)
            nc.sync.dma_start(
                out=out.rearrange("(p f) -> p f", p=32), in_=outsb
            )
            psT_cm.__exit__(None, None, None)
    return nc


_CACHE = {}
LAST_RESULTS = None


def _get_nc():
    if "nc" not in _CACHE:
        nc = bacc.Bacc("TRN2", target_bir_lowering=False, debug=False, num_devices=NCORES)
        _build_core_program(nc)
        nc.compile()
        _CACHE["nc"] = nc
    return _CACHE["nc"]


def _bf(v):
    return np.asarray(v, dtype=BF)


def _split2(v):
    hi = _bf(v)
    lo = _bf(np.asarray(v, np.float32) - hi.astype(np.float32))
    return hi, lo


def _host_prep(x, data, weights):
    lo = float(min(x.min(), data.min())) - PADG
    hi = float(max(x.max(), data.max())) + PADG
    ng = int(np.ceil((hi - lo) / H)) + 1
    assert ng <= NG, f"grid {ng} exceeds padded size {NG}"
    u = np.full(NG, -1000.0)
    u[:ng] = lo + np.arange(ng) * H

    w1, w1f = _split2(2.0 * A * u)
    v1, v1f = _split2(-A * u * u)
    # jcat layout: cols/partitions 0-63 = dim1 (q1 side), 64-127 = dim0
    m1 = np.zeros(128); m1[:64] = 1.0
    m0 = np.zeros(128); m0[64:] = 1.0
    w1c = np.tile(w1.astype(np.float64), 2)
    w1fc = np.tile(w1f.astype(np.float64), 2)
    v1c = np.tile(v1.astype(np.float64), 2)
    v1fc = np.tile(v1f.astype(np.float64), 2)
    ubias = np.tile(-A * u * u, 2).astype(np.float32)[:, None]

    # stage A: per-point aug rows, batched 4 chunks per matmul
    d0 = data[:, 0].astype(np.float64)
    d1 = data[:, 1].astype(np.float64)
    lnw = np.maximum(
        np.log(np.maximum(weights.astype(np.float64), 1e-300)), -200.0
    )
    d0c, d0f = _split2(d0)
    d1c, d1f = _split2(d1)
    e0h, e0l = _split2(-A * d0 * d0 + lnw)
    e1h, e1l = _split2(-A * d1 * d1)
    one = np.ones(NPTS)
    augA = np.stack(
        [d0c, d0f, d0c, e0h, e0l, d1c, d1f, d1c, e1h, e1l, one, one]
    ).astype(np.float32)  # [12, NPTS]
    uwa1 = np.stack([
        w1c * m0, w1c * m0, w1fc * m0, m0, m0,
        w1c * m1, w1c * m1, w1fc * m1, m1, m1,
        v1c, v1fc,
    ]).astype(np.float32)  # [12, 128]
    # augA4[12*s + q, m*128 + p] = augA[q, (4m+s)*128 + p]
    a = augA.reshape(KA, NPTS // 512, 4, 128)        # [q, m, s, p]
    augA4 = _bf(a.transpose(2, 0, 1, 3).reshape(KA4, NPTS // 4))  # cols: group-major
    # uwa4[12*s + q, 128*s + j] = uwa1[q, j], zero elsewhere
    uwa4 = np.zeros((KA4, 512), np.float32)
    for s in range(4):
        uwa4[s * KA : (s + 1) * KA, s * 128 : (s + 1) * 128] = uwa1
    uwa4 = _bf(uwa4)

    uwb = _bf(np.stack([
        w1c * m0, w1c * m0, w1fc * m0, m0, m0,
        w1c * m1, w1c * m1, w1fc * m1, m1, m1,
    ]))
    onesw = np.zeros((64, 32 * 32), np.float64)
    for t in range(32):
        onesw[:, 32 * t + t] = 1.0
    return augA4, uwa4, uwb, _bf(onesw), ubias


def _prep_augB(xsh):
    x0 = xsh[:, 0].astype(np.float64)
    x1 = xsh[:, 1].astype(np.float64)
    xc0, xf0 = _split2(x0)
    xc1, xf1 = _split2(x1)
    x2h0, x2l0 = _split2(-A * x0 * x0)
    x2h1, x2l1 = _split2(-A * x1 * x1)
    return _bf(np.stack(
        [xc0, xf0, xc0, x2h0, x2l0, xc1, xf1, xc1, x2h1, x2l1]
    ))


def kernel(x, data, weights):
    global LAST_RESULTS
    x = np.ascontiguousarray(x, dtype=np.float32)
    data = np.ascontiguousarray(data, dtype=np.float32)
    weights = np.ascontiguousarray(weights, dtype=np.float32)
    assert x.shape == (B, L, D) and data.shape == (NPTS, D)

    augA4, uwa4, uwb, onesw, ubias = _host_prep(x, data, weights)
    xf = x.reshape(NLOC, D)
    in_maps = []
    for c in range(NCORES):
        in_maps.append({
            "augA": augA4,
            "uwa": uwa4,
            "augB": _prep_augB(xf[c * NSH : (c + 1) * NSH]),
            "uwb": uwb,
            "ubias": ubias,
            "onesw": onesw,
        })

    nc = _get_nc()
    res = bass_utils.run_bass_kernel_spmd(
        nc, in_maps, core_ids=list(range(NCORES)),
        trace=bool(os.environ.get("BASS_TRACE")),
    )
    LAST_RESULTS = res
    out = np.concatenate([res.results[c]["out"] for c in range(NCORES)])
    return out.reshape(B, L)
